# revision 14
# baseline (speedup 1.0000x reference)
"""Trainium2 Bass kernel for nn_DictMoEDirect (moe_routing), v2.

Reference computation (fp32, shapes hardcoded):
  x = hidden_states.transpose(1,0,2)              # [B,S,H]
  g = mean_s(relu(x@gW1.T + gb1) @ gW2.T + gb2)   # [B,E]
  W1_b = sum_e g[b,e] eW1[e]; b1_b = g[b]@eb1     # per-sample merged MLP
  W2_b = sum_e g[b,e] eW2[e]; b2_b = g[b]@eb2
  y = relu(x@W1_b.T + b1_b) @ W2_b.T + b2_b       # [B,S,H]
  return y.transpose(1,0,2)                       # [S,B,H]

Distribution over 8 NeuronCores (v2):
  - Gate: data-parallel (core b computes g[b]), tiny AllGather of g.
  - Expert FFN: tensor-parallel over DFF (core j owns a 512-wide slice).
    The per-sample weight merges (sum_e g[b,e] eW[e]) are split by output
    column across THREE engines running concurrently:
      * PE  : diag(g_be)-matmul trick accumulating in PSUM (bf16, 1 cyc/row)
      * DVE : scalar_tensor_tensor chain (acc = ew*g + acc), fp32 accum
      * Pool: same on GpSimd
    while the PE also runs the actual GEMMs (bf16 operands, fp32 PSUM).
  - All tensors stored/moved in bf16 (tolerance 2e-2 allows it): halves
    DMA and SBUF. x and y1 stay resident in SBUF (no DRAM roundtrip).
  - Layer-2 partial sums are reduced across cores with two bf16
    ReduceScatters (one per H-half; the first overlaps second-half compute).

kernel(**inputs) takes full unsharded inputs, shards/transposes on the host,
runs the SPMD kernel, and reassembles the full [S,B,H] output.
"""

import numpy as np
import ml_dtypes

import concourse.bass as bass  # noqa: F401
import concourse.mybir as mybir
from concourse import bacc
from concourse.tile import TileContext
from concourse.masks import make_identity

H = 1024
DFF = 4096
E = 8
B = 8
S = 512
NC = 8
DSL = DFF // NC  # 512, per-core DFF slice
P = 128
F32 = mybir.dt.float32
F32R = mybir.dt.float32r
BF16 = mybir.dt.bfloat16
AF = mybir.ActivationFunctionType
ALU = mybir.AluOpType

# merge column split per engine (PE, DVE, Pool); phase1 splits the 256-wide
# o-half, phase2 the 512-wide h-half.  Pool share is 0: the neuronx
# backend rejects TensorScalarPtr on the GpSimd engine (ISA check).
P1_SPLIT = (184, 72, 0)
P2_SPLIT = (256, 256, 0)
assert sum(P1_SPLIT) == 256 and sum(P2_SPLIT) == 512


def build_module(debug=False, time_loop=0, time_phase=0, rs_bench=0, ag_bench=0, mock_ag=False):
    """time_loop=R wraps the FFN phases (not gate/collectives) in an
    on-device For loop for timing runs; outputs are then meaningless.
    rs_bench/ag_bench=K builds a collective-only module with K back-to-back
    ReduceScatters (bf16, kernel-sized) / AllGathers instead."""
    nc = bacc.Bacc()

    if rs_bench or ag_bench:
        y_out = nc.declare_dram_parameter("y2t", [H, S], BF16, isOutput=True)
        groups = [list(range(NC))]
        if rs_bench:
            rs_in = [nc.dram_tensor(f"rsb_in{i}", [B * 4 * P, S], BF16)
                     for i in range(rs_bench)]
            rs_out = [nc.dram_tensor(f"rsb_out{i}", [4 * P, S], BF16)
                      for i in range(rs_bench)]
        ag_in = [nc.dram_tensor(f"agb_in{i}", [E], F32) for i in range(ag_bench)]
        ag_out = [nc.dram_tensor(f"agb_out{i}", [NC * E], F32, addr_space="Shared")
                  for i in range(ag_bench)]
        with TileContext(nc) as tc:
            with tc.tile_pool(name="main", bufs=1) as pool:
                z = pool.tile([P, S], BF16, tag="z")
                nc.vector.memset(z[:], 0.0)
                zf = pool.tile([E, 1], F32, tag="zf")
                nc.vector.memset(zf[:], 0.0)
                for i in range(rs_bench):
                    for r in range(0, B * 4 * P, P):
                        nc.sync.dma_start(rs_in[i][r : r + P], z[:])
                for i in range(ag_bench):
                    nc.sync.dma_start(ag_in[i][:], zf[:, 0])
                # serialize successive collectives with a small data
                # dependency so K=9 vs K=1 differencing measures real time
                for i in range(rs_bench):
                    if i > 0:
                        nc.sync.dma_start(
                            rs_in[i][0:P], rs_out[i - 1][0:P]
                        )
                    nc.gpsimd.collective_compute(
                        "ReduceScatter", ALU.add, ins=[rs_in[i][:]],
                        outs=[rs_out[i][:]], replica_groups=groups)
                for i in range(ag_bench):
                    if i > 0:
                        nc.sync.dma_start(
                            ag_in[i][:], ag_out[i - 1][0:E]
                        )
                    nc.gpsimd.collective_compute(
                        "AllGather", ALU.bypass, ins=[ag_in[i][:]],
                        outs=[ag_out[i][:]], replica_groups=groups)
                if rs_bench:
                    nc.sync.dma_start(y_out[0 : 4 * P], rs_out[rs_bench - 1][:])
                else:
                    g_sb = pool.tile([1, NC * E], F32, tag="g")
                    nc.sync.dma_start(g_sb[:], ag_out[ag_bench - 1].ap()[None, :])
                    gb = pool.tile([1, NC * E], BF16, tag="gb")
                    nc.vector.tensor_copy(gb[:], g_sb[:])
                    nc.sync.dma_start(y_out[0, : NC * E], gb[0])
        nc.compile()
        return nc

    # ---- I/O ----
    xt_all = nc.declare_dram_parameter("xt_all", [B, H, S], BF16, isOutput=False)
    xt_own = nc.declare_dram_parameter("xt_own", [H, S], BF16, isOutput=False)
    gw1t = nc.declare_dram_parameter("gw1t", [H, H], BF16, isOutput=False)
    gb1t = nc.declare_dram_parameter("gb1t", [P, 8], F32, isOutput=False)
    gw2t = nc.declare_dram_parameter("gw2t", [H, E], BF16, isOutput=False)
    gb2 = nc.declare_dram_parameter("gb2", [E], F32, isOutput=False)
    ew1d = nc.declare_dram_parameter("ew1d", [2, 8, P, E, 256], BF16, isOutput=False)
    ew2d = nc.declare_dram_parameter("ew2d", [2, 4, P, E, 512], BF16, isOutput=False)
    eb1s = nc.declare_dram_parameter("eb1s", [E, DSL], F32R, isOutput=False)
    eb2 = nc.declare_dram_parameter("eb2", [E, H], F32R, isOutput=False)
    y_out = nc.declare_dram_parameter("y2t", [H, S], BF16, isOutput=True)
    if debug:
        dbg_g = nc.declare_dram_parameter("dbg_g", [NC * E], F32, isOutput=True)
        dbg_w1t = nc.declare_dram_parameter("dbg_w1t", [P, 8, 256], BF16, isOutput=True)
        dbg_y1 = nc.declare_dram_parameter("dbg_y1", [P, 4, S], BF16, isOutput=True)

    # ---- internal DRAM ----
    ag_in = nc.dram_tensor("ag_in", [E], F32)
    ag_out = nc.dram_tensor("ag_out", [NC * E], F32, addr_space="Shared")
    rs_in0 = nc.dram_tensor("rs_in0", [B, 4, P, S], BF16)
    rs_in1 = nc.dram_tensor("rs_in1", [B, 4, P, S], BF16)
    rs_out0 = nc.dram_tensor("rs_out0", [4 * P, S], BF16)
    rs_out1 = nc.dram_tensor("rs_out1", [4 * P, S], BF16)
    groups = [list(range(NC))]

    with TileContext(nc) as tc:
        with (
            tc.tile_pool(name="main", bufs=1) as pool,
            tc.tile_pool(name="psum", bufs=2, space="PSUM") as pp,
        ):
            # ---------------- gate (own sample) ----------------
            xo = pool.tile([P, 8, S], BF16, tag="xo")
            nc.sync.dma_start(
                xo[:], xt_own.rearrange("(k p) s -> p k s", p=P)
            )
            gb1_sb = pool.tile([P, 8], F32, tag="gb1")
            nc.sync.dma_start(gb1_sb[:], gb1t[:])
            h1 = pool.tile([P, 8, S], BF16, tag="h1")
            with tc.tile_pool(name="gatew", bufs=1) as gwpool:
                gw1_r = gwpool.tile([P, 8, H], BF16, tag="gw1")
                for k in range(8):
                    nc.sync.dma_start(
                        gw1_r[:, k], gw1t[k * P : (k + 1) * P, :]
                    )
                for m in range(8):
                    ps = pp.tile([P, S], F32, tag="out", bufs=3)
                    for k in range(8):
                        nc.tensor.matmul(
                            ps[:],
                            gw1_r[:, k, m * P : (m + 1) * P],
                            xo[:, k],
                            start=(k == 0),
                            stop=(k == 7),
                        )
                    nc.scalar.activation(
                        h1[:, m], ps[:], AF.Relu, bias=gb1_sb[:, m : m + 1]
                    )
                gw2_r = gwpool.tile([P, 8, E], BF16, tag="gw2")
                for k in range(8):
                    nc.sync.dma_start(
                        gw2_r[:, k], gw2t[k * P : (k + 1) * P, :]
                    )
                ps_g = pp.tile([E, S], F32, tag="tiny", bufs=1)
                for k in range(8):
                    nc.tensor.matmul(
                        ps_g[:],
                        gw2_r[:, k],
                        h1[:, k],
                        start=(k == 0),
                        stop=(k == 7),
                    )
                gsum = pool.tile([E, 1], F32, tag="gsum")
                nc.vector.reduce_sum(gsum[:], ps_g[:], axis=mybir.AxisListType.X)
                gb2_sb = pool.tile([E, 1], F32, tag="gb2")
                nc.sync.dma_start(gb2_sb[:], gb2[:, None])
                gmean = pool.tile([E, 1], F32, tag="gmean")
                nc.vector.tensor_scalar_mul(gmean[:], gsum[:], 1.0 / S)
                gown = pool.tile([E, 1], F32, tag="gown")
                nc.vector.tensor_add(gown[:], gmean[:], gb2_sb[:])
                nc.sync.dma_start(ag_in[:], gown[:, 0])

            if mock_ag:
                # timing-analysis build: no collectives (single-core CoreSim)
                for _c in range(NC):
                    nc.sync.dma_start(ag_out[_c * E : (_c + 1) * E], ag_in[:])
            else:
                nc.gpsimd.collective_compute(
                    "AllGather",
                    ALU.bypass,
                    ins=[ag_in[:]],
                    outs=[ag_out[:]],
                    replica_groups=groups,
                )
            if debug:
                nc.sync.dma_start(dbg_g[:], ag_out[:])

            # g broadcast across partitions [P, B*E]; transposed tiny [E, B]
            g_bc = pool.tile([P, NC * E], F32, tag="gbc")
            nc.sync.dma_start(
                g_bc[:], ag_out.ap()[None, :].broadcast_to([P, NC * E])
            )
            gT_r = pool.tile([E, B], F32R, tag="gT")
            nc.gpsimd.dma_start(gT_r[:], ag_out.rearrange("(b e) -> e b", e=E))

            # identity for the scaled-diag merge trick
            eye = pool.tile([P, P], F32, tag="eye")
            make_identity(nc, eye[:])

            # ---- merged per-sample biases ----
            # b1t[:, mt*8+b] = (g[b] @ eb1s)[mt-tile]      (full value)
            # b2t[:, m*8+b]  = (g[b] @ eb2)[m-tile] / 8    (1/8: summed by RS)
            eb1_r = pool.tile([E, DSL], F32R, tag="eb1")
            nc.sync.dma_start(eb1_r[:], eb1s[:])
            # eb2 arrives pre-scaled by 1/NC (host) as f32r
            eb2_r8 = pool.tile([E, H], F32R, tag="eb2r")
            nc.sync.dma_start(eb2_r8[:], eb2[:])
            b1t = pool.tile([P, 4 * B], F32, tag="b1t")
            b2t = pool.tile([P, 8 * B], F32, tag="b2t")
            for mt in range(4):
                ps = pp.tile([P, B], F32, tag="tiny", bufs=1)
                nc.tensor.matmul(
                    ps[:],
                    eb1_r[:, mt * P : (mt + 1) * P],
                    gT_r[:],
                    start=True,
                    stop=True,
                )
                nc.vector.tensor_copy(b1t[:, mt * B : (mt + 1) * B], ps[:])
            for m in range(8):
                ps = pp.tile([P, B], F32, tag="tiny", bufs=1)
                nc.tensor.matmul(
                    ps[:],
                    eb2_r8[:, m * P : (m + 1) * P],
                    gT_r[:],
                    start=True,
                    stop=True,
                )
                nc.vector.tensor_copy(b2t[:, m * B : (m + 1) * B], ps[:])

            # ---- per-(b,e) scaled identity tiles for the PE merge ----
            gd = {}
            for b in range(B):
                for e in range(E):
                    t = pool.tile([P, P], BF16, tag="gd", bufs=B * E)
                    nc.scalar.activation(
                        t[:], eye[:], AF.Copy,
                        scale=g_bc[:, b * E + e : b * E + e + 1],
                    )
                    gd[(b, e)] = t

            # ---- resident y1 (no DRAM roundtrip between phases) ----
            y1 = pool.tile([P, B, 4, S], BF16, tag="y1")

            def merge_split(w_t, ew_sb, b, nk, widths, psum_w):
                """w_t: [P, nk, W] dst; ew_sb: [P, nk, E, W] src; split W
                columns across PE/DVE/Pool.  PE part accumulates in PSUM
                (psum banks pack `per` k-tiles), DVE/Pool via stt chains with
                fp32 accumulators, bf16 final write."""
                w_pe, w_dve, w_pool = widths
                # --- PE ---
                per = 512 // w_pe  # k-tiles per PSUM bank
                for k0 in range(0, nk, per):
                    kn = min(per, nk - k0)
                    ps = pp.tile([P, 512], F32, tag="mg", bufs=4)
                    for k in range(k0, k0 + kn):
                        off = (k - k0) * w_pe
                        for e in range(E):
                            nc.tensor.matmul(
                                ps[:, off : off + w_pe],
                                gd[(b, e)][:],
                                ew_sb[:, k, e, :w_pe],
                                start=(e == 0),
                                stop=(e == E - 1),
                            )
                    nc.scalar.activation(
                        w_t[:, k0 : k0 + kn, :w_pe],
                        ps[:, : kn * w_pe].rearrange("p (k o) -> p k o", o=w_pe),
                        AF.Copy,
                    )
                # --- DVE / Pool ---
                for eng, lo, wd, acc in (
                    (nc.vector, w_pe, w_dve, psum_w[0]),
                    (nc.gpsimd, w_pe + w_dve, w_pool, psum_w[1]),
                ):
                    if wd == 0:
                        continue
                    hi = lo + wd
                    for e in range(E):
                        src = ew_sb[:, :, e, lo:hi]
                        if e == 0:
                            eng.scalar_tensor_tensor(
                                acc[:, :nk, :wd], src,
                                g_bc[:, b * E : b * E + 1], src,
                                ALU.mult, ALU.bypass,
                            )
                        elif e < E - 1:
                            eng.scalar_tensor_tensor(
                                acc[:, :nk, :wd], src,
                                g_bc[:, b * E + e : b * E + e + 1],
                                acc[:, :nk, :wd], ALU.mult, ALU.add,
                            )
                        else:
                            eng.scalar_tensor_tensor(
                                w_t[:, :, lo:hi], src,
                                g_bc[:, b * E + e : b * E + e + 1],
                                acc[:, :nk, :wd], ALU.mult, ALU.add,
                            )

            def phase1():
                HF = 256
                for p in range(2):
                    ew_sb = pool.tile([P, 8, E, HF], BF16, tag="ew", bufs=2)
                    for k in range(8):
                        nc.sync.dma_start(ew_sb[:, k], ew1d[p, k])
                    state = {}

                    def merge1(b):
                        xb = pool.tile([P, 8, S], BF16, tag="xb", bufs=3)
                        nc.sync.dma_start(
                            xb[:],
                            xt_all.rearrange("b (k p) s -> b p k s", p=P)[b],
                        )
                        w1t = pool.tile([P, 8, HF], BF16, tag="wmt", bufs=5)
                        acc_d = pool.tile(
                            [P, 8, P1_SPLIT[1]], F32, tag="acd", bufs=2
                        )
                        acc_p = None
                        merge_split(w1t, ew_sb, b, 8, P1_SPLIT, (acc_d, acc_p))
                        state[b] = (w1t, xb)

                    def gemm1(b):
                        w1t, xb = state.pop(b)
                        if debug and p == 0 and b == 0:
                            nc.sync.dma_start(dbg_w1t[:], w1t[:])
                        for m in range(2):
                            mt = p * 2 + m
                            ps = pp.tile([P, S], F32, tag="out", bufs=3)
                            for k in range(8):
                                nc.tensor.matmul(
                                    ps[:],
                                    w1t[:, k, m * P : (m + 1) * P],
                                    xb[:, k],
                                    start=(k == 0),
                                    stop=(k == 7),
                                )
                            nc.scalar.activation(
                                y1[:, b, mt],
                                ps[:],
                                AF.Relu,
                                bias=b1t[:, mt * B + b : mt * B + b + 1],
                            )

                    for b in range(B + 1):
                        if b < B:
                            merge1(b)
                        if b >= 1:
                            gemm1(b - 1)

            def phase2(with_rs=True):
                HH = 512
                for p in range(2):
                    rs_in = rs_in0 if p == 0 else rs_in1
                    ew_sb = pool.tile([P, 4, E, HH], BF16, tag="ew", bufs=2)
                    for kt in range(4):
                        nc.sync.dma_start(ew_sb[:, kt], ew2d[p, kt])
                    state2 = {}

                    def merge2(b):
                        w2t = pool.tile([P, 4, HH], BF16, tag="wmt", bufs=5)
                        acc_d = pool.tile(
                            [P, 4, P2_SPLIT[1]], F32, tag="acd", bufs=2
                        )
                        acc_p = None
                        merge_split(w2t, ew_sb, b, 4, P2_SPLIT, (acc_d, acc_p))
                        state2[b] = w2t

                    def gemm2(b):
                        w2t = state2.pop(b)
                        for m in range(4):
                            mg = p * 4 + m
                            ps = pp.tile([P, S], F32, tag="out", bufs=3)
                            for kt in range(4):
                                nc.tensor.matmul(
                                    ps[:],
                                    w2t[:, kt, m * P : (m + 1) * P],
                                    y1[:, b, kt],
                                    start=(kt == 0),
                                    stop=(kt == 3),
                                )
                            y2 = pool.tile([P, S], BF16, tag="y2", bufs=4)
                            nc.scalar.activation(
                                y2[:],
                                ps[:],
                                AF.Identity,
                                bias=b2t[:, mg * B + b : mg * B + b + 1],
                            )
                            nc.sync.dma_start(rs_in[b, m], y2[:])

                    for b in range(B + 1):
                        if b < B:
                            merge2(b)
                        if b >= 1:
                            gemm2(b - 1)

                    if with_rs:
                        nc.gpsimd.collective_compute(
                            "ReduceScatter",
                            ALU.add,
                            ins=[rs_in.ap().rearrange("b m p s -> (b m p) s")],
                            outs=[(rs_out0 if p == 0 else rs_out1)[:]],
                            replica_groups=groups,
                        )

            if time_loop:
                with tc.For_i(0, time_loop, 1):
                    if time_phase in (0, 1):
                        phase1()
                    if time_phase in (0, 2):
                        phase2(with_rs=False)
                nc.sync.dma_start(
                    y_out[0 : 4 * P], rs_in0.ap().rearrange("b m p s -> (b m p) s")[0 : 4 * P]
                )
                nc.sync.dma_start(
                    y_out[4 * P : 8 * P],
                    rs_in1.ap().rearrange("b m p s -> (b m p) s")[0 : 4 * P],
                )
            else:
                phase1()
                phase2(with_rs=True)
                if debug:
                    nc.sync.dma_start(dbg_y1[:], y1[:, 0])
                nc.sync.dma_start(y_out[0 : 4 * P], rs_out0[:])
                nc.sync.dma_start(y_out[4 * P : 8 * P], rs_out1[:])

    nc.compile()
    return nc


def _bf(a):
    return np.asarray(a, np.float32).astype(ml_dtypes.bfloat16)


def _ew1_dev(a):
    # a: [E, DSL(o), H(i)] -> [2pass, 8k, 128p(i), 8e, 256o] bf16
    a2 = np.ascontiguousarray(np.asarray(a, np.float32).transpose(2, 0, 1))
    a3 = a2.reshape(8, P, E, 2, 256).transpose(3, 0, 1, 2, 4)
    return _bf(np.ascontiguousarray(a3))


def _ew2_dev(c):
    # c: [E, H(h), DSL(d)] -> [2pass, 4kt, 128p(d), 8e, 512h] bf16
    c2 = np.ascontiguousarray(np.asarray(c, np.float32).transpose(2, 0, 1))
    c3 = c2.reshape(4, P, E, 2, 512).transpose(3, 0, 1, 2, 4)
    return _bf(np.ascontiguousarray(c3))


def _shard_inputs(hidden_states, gW1, gb1, gW2, gb2, eW1, eb1, eW2, eb2):
    xt_all = _bf(
        np.ascontiguousarray(
            np.asarray(hidden_states, dtype=np.float32).transpose(1, 2, 0)
        )
    )  # [B, H, S] bf16
    gW1t = _bf(np.ascontiguousarray(np.asarray(gW1, np.float32).T))
    gb1t = np.ascontiguousarray(np.asarray(gb1, np.float32).reshape(8, P).T)
    gW2t = _bf(np.ascontiguousarray(np.asarray(gW2, np.float32).T))
    gb2 = np.ascontiguousarray(np.asarray(gb2, np.float32))
    eW1 = np.asarray(eW1, np.float32)
    eW2 = np.asarray(eW2, np.float32)
    eb1 = np.asarray(eb1, np.float32)
    eb2 = np.ascontiguousarray(np.asarray(eb2, np.float32) / NC)
    in_maps = []
    for j in range(NC):
        sl = slice(j * DSL, (j + 1) * DSL)
        in_maps.append(
            {
                "xt_all": xt_all,
                "xt_own": np.ascontiguousarray(xt_all[j]),
                "gw1t": gW1t,
                "gb1t": gb1t,
                "gw2t": gW2t,
                "gb2": gb2,
                "ew1d": _ew1_dev(eW1[:, sl, :]),
                "ew2d": _ew2_dev(eW2[:, :, sl]),
                "eb1s": np.ascontiguousarray(eb1[:, sl]),
                "eb2": eb2,
            }
        )
    return in_maps


# ---------------- SPMD runner (persistent jit over axon PJRT) -----------

_CACHE = {}


def _build_runner(debug=False, time_loop=0, time_phase=0, rs_bench=0, ag_bench=0):
    import jax
    from jax.sharding import Mesh, PartitionSpec
    from jax.experimental.shard_map import shard_map
    from concourse import bass2jax

    nc = build_module(
        debug=debug, time_loop=time_loop, time_phase=time_phase,
        rs_bench=rs_bench, ag_bench=ag_bench,
    )
    bass2jax.install_neuronx_cc_hook()
    partition_name = nc.partition_id_tensor.name if nc.partition_id_tensor else None

    in_names, out_names, out_avals = [], [], []
    for alloc in nc.m.functions[0].allocations:
        if not isinstance(alloc, mybir.MemoryLocationSet):
            continue
        name = alloc.memorylocations[0].name
        if alloc.kind == "ExternalInput":
            if name != partition_name:
                in_names.append(name)
        elif alloc.kind == "ExternalOutput":
            out_avals.append(
                jax.core.ShapedArray(
                    tuple(alloc.tensor_shape), mybir.dt.np(alloc.dtype)
                )
            )
            out_names.append(name)
    n_outs = len(out_names)
    all_in_names = list(in_names) + list(out_names)
    if partition_name is not None:
        all_in_names.append(partition_name)

    def _body(*args):
        operands = list(args)
        if partition_name is not None:
            operands.append(bass2jax.partition_id_tensor())
        return tuple(
            bass2jax._bass_exec_p.bind(
                *operands,
                out_avals=tuple(out_avals),
                in_names=tuple(all_in_names),
                out_names=tuple(out_names),
                lowering_input_output_aliases=(),
                sim_require_finite=True,
                sim_require_nnan=True,
                nc=nc,
            )
        )

    devices = jax.devices()[:NC]
    mesh = Mesh(np.asarray(devices), ("core",))
    n_params = len(in_names)
    sharded = jax.jit(
        shard_map(
            _body,
            mesh=mesh,
            in_specs=(PartitionSpec("core"),) * (n_params + n_outs),
            out_specs=(PartitionSpec("core"),) * n_outs,
            check_rep=False,
        ),
        keep_unused=True,
    )
    zero_shapes = [((NC * a.shape[0], *a.shape[1:]), a.dtype) for a in out_avals]

    def run(in_maps, device_inputs=None, fetch=True):
        if device_inputs is None:
            concat_in = [
                np.concatenate(
                    [np.asarray(in_maps[c][n]) for c in range(NC)], axis=0
                )
                for n in in_names
            ]
            dev_params = [jax.device_put(x) for x in concat_in]
            dev_zeros = [jax.device_put(np.zeros(s, d)) for s, d in zero_shapes]
            device_inputs = (dev_params, dev_zeros)
            jax.block_until_ready(dev_params)
            jax.block_until_ready(dev_zeros)
        dev_params, dev_zeros = device_inputs
        out_arrs = sharded(*dev_params, *dev_zeros)
        jax.block_until_ready(out_arrs)
        if not fetch:
            return None, device_inputs
        results = [
            {
                name: np.asarray(out_arrs[i]).reshape(NC, *out_avals[i].shape)[c]
                for i, name in enumerate(out_names)
            }
            for c in range(NC)
        ]
        return results, device_inputs

    return run


def get_runner(debug=False, time_loop=0, time_phase=0, rs_bench=0, ag_bench=0):
    key = ("run", debug, time_loop, time_phase, rs_bench, ag_bench)
    if key not in _CACHE:
        _CACHE[key] = _build_runner(
            debug=debug, time_loop=time_loop, time_phase=time_phase,
            rs_bench=rs_bench, ag_bench=ag_bench,
        )
    return _CACHE[key]


def kernel(**inputs) -> np.ndarray:
    run = get_runner()
    in_maps = _shard_inputs(**inputs)
    results, _ = run(in_maps)
    # core b's output is y2^T[b] = [H, S] bf16; assemble [S, B, H] fp32
    y2t = np.stack(
        [results[b]["y2t"].astype(np.float32) for b in range(B)], axis=0
    )  # [B, H, S]
    return np.ascontiguousarray(y2t.transpose(2, 0, 1)).astype(np.float32)
